# revision 1
# baseline (speedup 1.0000x reference)
"""COIL scoring kernel for Trainium2, sharded over 8 NeuronCores.

Sharding: data-parallel over documents (Bd=256 -> 32 docs/core). Each core:
  - projects its doc tokens with W_tok (+ReLU) and token-0 with W_cls
  - projects the (replicated) query tokens the same way
  - computes token-level scores via one [97 x 128 x 512] matmul family where
    the exact-match id constraint is encoded as extra one-hot "digit"
    dimensions (base-32 digits of the id, scaled by 64) plus a -8192 penalty
    constant: equal ids add 0 to the score, any mismatch adds <= -4096, so
    relu(max_j score) == max_j (score * exact_match) for this data regime.
  - reduces max over doc tokens per doc, then folds the query-token mask
    (sep masking + drop i=0) via a selector matmul, accumulated on top of the
    CLS score matmul in the same PSUM tile.
Host: builds bf16/transposed layouts, gathers per-core [32,32] score tiles,
computes the softmax loss in numpy.
"""

import os
import numpy as np
import ml_dtypes

import concourse.bass as bass
import concourse.bacc as bacc
import concourse.mybir as mybir
from concourse import tile
from concourse.bass_utils import run_bass_kernel_spmd

BF16 = mybir.dt.bfloat16
F32 = mybir.dt.float32
RELU = mybir.ActivationFunctionType.Relu
IDENT = mybir.ActivationFunctionType.Identity
AX_X = mybir.AxisListType.X
PSUM = bass.MemorySpace.PSUM

N_CORES = 8
Bq, Lq, Bd, Ld, H, Dt, Dc = 32, 32, 256, 128, 768, 32, 768
DPC = Bd // N_CORES          # docs per core = 32
NDT = DPC * Ld               # doc tokens per core = 4096
NQT = Bq * Lq                # query tokens = 1024
KC = H // 128                # contraction chunks = 6
EXT = 65                     # id-encoding extension rows (32+32 digits + const)
KTOT = Dt + EXT              # 97
CONE = 64.0                  # one-hot scale; CONE^2 = 4096 penalty unit
PEN = -2.0 * CONE * CONE     # -8192
NT = NQT // 128              # query-token tiles = 8
NN = NDT // 512              # doc-token 512-chunks = 8
TRAIN_GROUP_SIZE = 8

bf16 = ml_dtypes.bfloat16

_prog = None
last_results = None          # BassKernelResults of the most recent run


def _build():
    nc = bacc.Bacc("TRN2", target_bir_lowering=False, debug=False)

    dhT_d = nc.dram_tensor("dhT", [H, NDT], BF16, kind="ExternalInput")
    qhT_d = nc.dram_tensor("qhT", [H, NQT], BF16, kind="ExternalInput")
    h0T_d = nc.dram_tensor("h0T", [H, 2 * Bq], BF16, kind="ExternalInput")
    wtok_d = nc.dram_tensor("wtok", [H, Dt], BF16, kind="ExternalInput")
    wcls_d = nc.dram_tensor("wcls", [H, Dc], BF16, kind="ExternalInput")
    btok_d = nc.dram_tensor("btok", [Dt, 1], F32, kind="ExternalInput")
    bcls_d = nc.dram_tensor("bcls", [Dc, 1], F32, kind="ExternalInput")
    extq_d = nc.dram_tensor("extq", [EXT, NQT], BF16, kind="ExternalInput")
    extd_d = nc.dram_tensor("extd", [EXT, NDT], BF16, kind="ExternalInput")
    wsel_d = nc.dram_tensor("wsel", [NQT, Bq], BF16, kind="ExternalInput")
    out_d = nc.dram_tensor("scores_out", [Bq, DPC], F32, kind="ExternalOutput")

    with tile.TileContext(nc) as tc:
        with (
            tc.tile_pool(name="big", bufs=1) as bigp,
            tc.tile_pool(name="psP", bufs=2, space=PSUM) as psPp,
            tc.tile_pool(name="psS", bufs=3, space=PSUM) as psSp,
            tc.tile_pool(name="psC", bufs=2, space=PSUM) as psCp,
            tc.tile_pool(name="psF", bufs=1, space=PSUM) as psFp,
        ):
            # ---- constants / small loads
            wtok_sb = bigp.tile([128, KC, Dt], BF16, tag="wtok")
            wcls_sb = bigp.tile([128, KC, Dc], BF16, tag="wcls")
            h0T_sb = bigp.tile([128, KC, 2 * Bq], BF16, tag="h0T")
            btok_sb = bigp.tile([Dt, 1], F32, tag="btok")
            bcls_sb = bigp.tile([128, KC], F32, tag="bcls")
            wsel_sb = bigp.tile([128, NT, Bq], BF16, tag="wsel")
            for c in range(KC):
                sl = slice(c * 128, (c + 1) * 128)
                nc.sync.dma_start(out=wtok_sb[:, c, :], in_=wtok_d[sl, :])
                nc.sync.dma_start(out=wcls_sb[:, c, :], in_=wcls_d[sl, :])
                nc.sync.dma_start(out=h0T_sb[:, c, :], in_=h0T_d[sl, :])
                nc.sync.dma_start(out=bcls_sb[:, c : c + 1], in_=bcls_d[sl, :])
            nc.sync.dma_start(out=btok_sb[:, :], in_=btok_d[:, :])
            for t in range(NT):
                nc.sync.dma_start(
                    out=wsel_sb[:, t, :], in_=wsel_d[t * 128 : (t + 1) * 128, :]
                )

            # ---- query side: QT[0:97, 1024] = [qry_reps^T ; ext_q]
            QT = bigp.tile([128, NQT], BF16, tag="QT")
            nc.sync.dma_start(out=QT[Dt : Dt + EXT, :], in_=extq_d[:, :])
            qhT_sb = bigp.tile([128, KC, NQT], BF16, tag="qhT")
            for c in range(KC):
                nc.sync.dma_start(
                    out=qhT_sb[:, c, :], in_=qhT_d[c * 128 : (c + 1) * 128, :]
                )
            for h in range(2):
                ps = psPp.tile([Dt, 512], F32, tag="proj")
                hs = slice(h * 512, (h + 1) * 512)
                for c in range(KC):
                    nc.tensor.matmul(
                        ps[:, :],
                        lhsT=wtok_sb[:, c, :],
                        rhs=qhT_sb[:, c, hs],
                        start=(c == 0),
                        stop=(c == KC - 1),
                    )
                nc.scalar.activation(QT[0:Dt, hs], ps[:, :], RELU, bias=btok_sb[:, 0:1])

            # ---- cls projections: qdcls[:, m, 0:32]=qry_cls^T chunk, [:,m,32:64]=doc_cls^T
            qdcls = bigp.tile([128, KC, 2 * Bq], BF16, tag="qdcls")
            for m in range(KC):
                psc = psCp.tile([128, 2 * Bq], F32, tag="cls")
                for c in range(KC):
                    nc.tensor.matmul(
                        psc[:, :],
                        lhsT=wcls_sb[:, c, m * 128 : (m + 1) * 128],
                        rhs=h0T_sb[:, c, :],
                        start=(c == 0),
                        stop=(c == KC - 1),
                    )
                nc.scalar.activation(
                    qdcls[:, m, :], psc[:, :], IDENT, bias=bcls_sb[:, m : m + 1]
                )

            # ---- doc side: stream per 1024-col quarter, project into DT chunks
            DT_tiles = []
            for n in range(NN):
                DT_n = bigp.tile([128, 512], BF16, tag=f"DT{n}")
                DT_tiles.append(DT_n)
                nc.sync.dma_start(
                    out=DT_n[Dt : Dt + EXT, :], in_=extd_d[:, n * 512 : (n + 1) * 512]
                )
            dhq = []
            for q in range(4):
                dq = bigp.tile([128, KC, 1024], BF16, tag=f"dh{q}")
                dhq.append(dq)
                for c in range(KC):
                    nc.sync.dma_start(
                        out=dq[:, c, :],
                        in_=dhT_d[c * 128 : (c + 1) * 128, q * 1024 : (q + 1) * 1024],
                    )
            for n in range(NN):
                q, half = n // 2, n % 2
                ps = psPp.tile([Dt, 512], F32, tag="proj")
                hs = slice(half * 512, (half + 1) * 512)
                for c in range(KC):
                    nc.tensor.matmul(
                        ps[:, :],
                        lhsT=wtok_sb[:, c, :],
                        rhs=dhq[q][:, c, hs],
                        start=(c == 0),
                        stop=(c == KC - 1),
                    )
                nc.scalar.activation(
                    DT_tiles[n][0:Dt, :], ps[:, :], RELU, bias=btok_sb[:, 0:1]
                )

            # ---- token scores: S = QT^T @ DT (K=97), masked max via penalty + relu
            tokraw = bigp.tile([128, NT, DPC], F32, tag="tokraw")
            tok = bigp.tile([128, NT, DPC], BF16, tag="tok")
            for t in range(NT):
                for n in range(NN):
                    psS = psSp.tile([128, 512], F32, tag="S")
                    nc.tensor.matmul(
                        psS[:, :],
                        lhsT=QT[0:KTOT, t * 128 : (t + 1) * 128],
                        rhs=DT_tiles[n][0:KTOT, :],
                        start=True,
                        stop=True,
                    )
                    nc.vector.reduce_max(
                        tokraw[:, t, n * 4 : (n + 1) * 4],
                        psS[:, :].rearrange("p (a b) -> p a b", b=Ld),
                        axis=AX_X,
                    )
                nc.scalar.activation(tok[:, t, :], tokraw[:, t, :], RELU)

            # ---- final: scores = cls + sum_i w[a,i] * tok, all in one PSUM accum
            psF = psFp.tile([Bq, DPC], F32, tag="fin")
            for m in range(KC):
                nc.tensor.matmul(
                    psF[:, :],
                    lhsT=qdcls[:, m, 0:Bq],
                    rhs=qdcls[:, m, Bq : 2 * Bq],
                    start=(m == 0),
                    stop=False,
                )
            for t in range(NT):
                nc.tensor.matmul(
                    psF[:, :],
                    lhsT=wsel_sb[:, t, :],
                    rhs=tok[:, t, :],
                    start=False,
                    stop=(t == NT - 1),
                )
            scr = bigp.tile([Bq, DPC], F32, tag="scr")
            nc.vector.tensor_copy(scr[:, :], psF[:, :])
            nc.sync.dma_start(out=out_d[:, :], in_=scr[:, :])

    nc.compile()
    return nc


def _get_prog():
    global _prog
    if _prog is None:
        _prog = _build()
    return _prog


def _prep_inputs(qry_hidden, doc_hidden, W_tok, b_tok, W_cls, b_cls,
                 qry_input_ids, doc_input_ids, qry_attention_mask):
    qh = np.asarray(qry_hidden, np.float32)
    dh = np.asarray(doc_hidden, np.float32)
    qids = np.asarray(qry_input_ids, np.int32).reshape(-1)
    dids = np.asarray(doc_input_ids, np.int32)
    amask = np.asarray(qry_attention_mask, np.int32)

    qhT = np.ascontiguousarray(qh.reshape(NQT, H).astype(bf16).T)
    wtok = np.asarray(W_tok, np.float32).astype(bf16)
    wcls = np.asarray(W_cls, np.float32).astype(bf16)
    btok = np.asarray(b_tok, np.float32).reshape(Dt, 1)
    bcls = np.asarray(b_cls, np.float32).reshape(Dc, 1)

    g = np.arange(NQT)
    extq = np.zeros((EXT, NQT), np.float32)
    extq[qids % 32, g] = CONE
    extq[32 + qids // 32, g] = CONE
    extq[64, :] = 1.0
    extq = extq.astype(bf16)

    # query-token weights: qmask with sep position zeroed, i=0 dropped
    sep = amask.sum(1) - 1
    qm = amask.astype(np.float32).copy()
    qm[np.arange(Bq), sep] = 0.0
    w = qm.copy()
    w[:, 0] = 0.0
    wsel = np.zeros((NQT, Bq), np.float32)
    wsel[g, g // Lq] = w.reshape(-1)
    wsel = wsel.astype(bf16)

    qh0 = qh[:, 0, :]
    in_maps = []
    for k in range(N_CORES):
        dsl = slice(k * DPC, (k + 1) * DPC)
        dh_k = dh[dsl].reshape(NDT, H)
        dhT_k = np.ascontiguousarray(dh_k.astype(bf16).T)
        h0T_k = np.ascontiguousarray(
            np.concatenate([qh0, dh[dsl, 0, :]], axis=0).astype(bf16).T
        )
        dids_k = dids[dsl].reshape(-1)
        gd = np.arange(NDT)
        extd = np.zeros((EXT, NDT), np.float32)
        extd[dids_k % 32, gd] = CONE
        extd[32 + dids_k // 32, gd] = CONE
        extd[64, :] = PEN
        in_maps.append({
            "dhT": dhT_k,
            "qhT": qhT,
            "h0T": h0T_k,
            "wtok": wtok,
            "wcls": wcls,
            "btok": btok,
            "bcls": bcls,
            "extq": extq,
            "extd": extd.astype(bf16),
            "wsel": wsel,
        })
    return in_maps


def kernel(**inputs):
    global last_results
    nc = _get_prog()
    in_maps = _prep_inputs(**inputs)
    trace = bool(os.environ.get("COIL_TRACE"))
    last_results = run_bass_kernel_spmd(
        nc, in_maps, list(range(N_CORES)), trace=trace
    )
    scores = np.concatenate(
        [last_results.results[k]["scores_out"] for k in range(N_CORES)], axis=1
    ).astype(np.float64)

    labels = np.arange(Bq) * TRAIN_GROUP_SIZE
    m = scores.max(axis=1, keepdims=True)
    lse = m[:, 0] + np.log(np.exp(scores - m).sum(axis=1))
    loss = -(scores[np.arange(Bq), labels] - lse).mean()
    return (
        np.asarray(loss, np.float32),
        scores.reshape(-1).astype(np.float32),
    )
